# revision 29
# baseline (speedup 1.0000x reference)
"""Multi-head attention + out-proj + residual + LayerNorm on 8 trn2 cores.

Sharding: (batch, seq-half) -> 8 shards, collective-free. Each core gets
transposed activations plus shared weights and computes its full
[1024, 1024] output block.  Measured ~421us (baseline 741us).

Single fused pipeline, everything SBUF-resident (no DRAM staging):
  - qT/kT/vT and Wq/Wk/Wv stored fp8e4, weights pre-scaled x16 on the
    host (fp8e4m3 subnormal avoidance); projections run as fp8 DoubleRow
    matmuls (pair-interleaved [p, c2, j, :] layout, contraction 256/pass).
  - projection work (K_c, Q_c, V chunks) is emitted through a generator
    and interleaved into the attention inner loop, paced one head-pair
    chunk ahead of consumption (emitting a chunk too early recycles a kt
    pool slot whose reader scores are not yet emitted -> scheduler
    deadlock; too late starves the PE).
  - per head-pair chunk c: scoresT[k,q] = KT_c.T @ QT_c (the two heads
    ride disjoint PE row groups via tile_position), exp on ACT with the
    1/(sqrt(D)*256) fold, OT[dv+1,q] += [V|1].T @ exp (row 64 = softmax
    denominator).  Scores for the next block's chunk 0 are prefetched at
    iter 14 so the ACT stream does not break at block boundaries.
  - normalize, entirely off the PE: one quick PSUM->SBUF drain frees the
    accumulator bank (~0.7us, keeps the PE from bubbling >3.4us at block
    boundaries, which would re-throttle the PE clock to 1.2 GHz);
    denominator hops to partition 0 via a partition-shifted DVE copy
    (the one shifted op that works; reciprocal_approx_fast silently
    ignores nonzero base partitions), reciprocal_approx_fast, 1/16
    unscale, gpsimd partition_broadcast, TT multiply.  hh=1 lands on
    partitions 64:127 via a second shifted copy.
  - final: out = LN(ot_sb.T @ Wp + qres)*scale + offset; the residual is
    accumulated into the out-proj PSUM via an identity matmul on a bf16
    copy of q, and (x-mean)*gamma is fused in one scalar_tensor_tensor.

Hardware landmines found on the way (verified by experiment):
  - a 1-contraction (32,64)@(64,0) broadcast matmul emitted inside an
    open PSUM accumulation window corrupts other in-flight matmul groups;
  - reciprocal_approx_fast returns garbage for APs based at partition!=0;
  - DVE tensor_tensor with mismatched operand base partitions does not
    compile; partition-shifted tensor_copy works and is bit-exact.
"""

import os
from contextlib import ExitStack

import numpy as np

import concourse.bass as bass
import concourse.tile as tile
from concourse import bacc, mybir
from concourse._compat import with_exitstack
from concourse.bass_utils import run_bass_kernel_spmd

B, S, D = 4, 2048, 1024
H, DK, DV = 16, 64, 64
F = H * DV            # 1024 flattened head dim (== H*DK)
N_CORES = 8
SQ = S // 2           # 1024 queries per core
SK = S                # 2048 keys per core
P = 128
KD = D // P           # 8 contraction chunks over d_model
NF = F // P           # 8 head-pair chunks
NSK = SK // P         # 16 key chunks
TEMP = float(np.sqrt(D))
WS = 16.0             # host-side weight scale (fp8 subnormal avoidance)
EXP_SCALE = 1.0 / (TEMP * WS * WS)
EPS = 1e-9

F32 = mybir.dt.float32
BF16 = mybir.dt.bfloat16
FP8 = mybir.dt.float8e4
I16 = mybir.dt.int16
DR = mybir.MatmulPerfMode.DoubleRow
# Schraudolph fast-exp constants (bf16 bit trick): round(x*128/ln2 + bias)
# interpreted as bf16 bits ~= exp(x) to ~2-4% -- plenty within tolerance,
# used on a few chunks per block to offload the ACT engine.
SCHRA_A = EXP_SCALE * 128.0 / float(np.log(2.0))
SCHRA_B = 127.0 * 128.0 - 4.5
DVE_EXP_SKS = ()

LAST_RESULT = None


@with_exitstack
def _mha_kernel(ctx: ExitStack, tc: tile.TileContext, out_ap, ins):
    nc = tc.nc
    AF = mybir.ActivationFunctionType
    ALU = mybir.AluOpType

    g_const = ctx.enter_context(tc.tile_pool(name="gconst", bufs=1))
    ones_sb = g_const.tile([P, 64], BF16)
    nc.vector.memset(ones_sb, 1.0)
    # warm-up fodder: keeps the PE HAM busy through the initial DMA wait
    # (else the first ~20us of real matmuls run at 1.2 GHz) and triggers
    # the ACT exp table load (~2.7us) before the first real exp.
    warm_b = g_const.tile([P, 512], BF16)
    nc.vector.memset(warm_b, 0.125)
    warm_f = g_const.tile([P, 64], F32)
    nc.vector.memset(warm_f, 0.0)
    warm_e = g_const.tile([P, 64], BF16)
    nc.scalar.activation(warm_e, warm_f, mybir.ActivationFunctionType.Exp,
                         scale=1.0)

    res = ctx.enter_context(tc.tile_pool(name="resident", bufs=1))
    # inputs (fp8) in DoubleRow pair-interleaved layout: D-row
    # c2*256 + 2p + j lives at [p, c2, j, :]; loaded in pieces so early
    # matmuls unblock.
    KD2 = KD // 2
    xq = res.tile([P, KD2, 2, SQ], FP8)
    xk = res.tile([P, KD2, 2, SK], FP8)
    xv = res.tile([P, KD2, 2, SK], FP8)
    wq = res.tile([P, KD2, 2, F], FP8)
    wk = res.tile([P, KD2, 2, F], FP8)
    wv = res.tile([P, KD2, 2, F], FP8)
    wp = res.tile([P, NF, D], BF16)
    xq_r = ins["qT"].rearrange("(c p j) s -> p c j s", c=KD2, j=2)
    xk_r = ins["kT"].rearrange("(c p j) s -> p c j s", c=KD2, j=2)
    xv_r = ins["vT"].rearrange("(c p j) s -> p c j s", c=KD2, j=2)
    wq_r = ins["wqT"].rearrange("(c p j) f -> p c j f", c=KD2, j=2)
    wk_r = ins["wkT"].rearrange("(c p j) f -> p c j f", c=KD2, j=2)
    wv_r = ins["wvT"].rearrange("(c p j) f -> p c j f", c=KD2, j=2)
    # K-proj inputs first (dma_starts spread round-robin over the DMA
    # queues land in parallel), then Q, V, and the late-needed wp.
    for c2 in range(KD2):
        for j in range(2):
            nc.sync.dma_start(wk[:, c2, j, :], wk_r[:, c2, j, :])
            nc.sync.dma_start(xk[:, c2, j, 0:SK // 2],
                              xk_r[:, c2, j, 0:SK // 2])
            nc.sync.dma_start(xk[:, c2, j, SK // 2:],
                              xk_r[:, c2, j, SK // 2:])
    for c2 in range(KD2):
        for j in range(2):
            nc.sync.dma_start(wq[:, c2, j, :], wq_r[:, c2, j, :])
            nc.sync.dma_start(xq[:, c2, j, :], xq_r[:, c2, j, :])
    for c2 in range(KD2):
        for j in range(2):
            nc.sync.dma_start(wv[:, c2, j, :], wv_r[:, c2, j, :])
            nc.sync.dma_start(xv[:, c2, j, :], xv_r[:, c2, j, :])
    wp_r = ins["wpT"].rearrange("(c p) f -> p c f", p=P)
    for f_ in range(NF):
        nc.sync.dma_start(wp[:, f_, :], wp_r[:, f_, :])
    ident = res.tile([P, P], BF16)
    nc.sync.dma_start(ident, ins["ident"])

    # persistent activations
    v_sb = res.tile([P, NSK, H, 65], BF16)   # [V | ones] per head
    nc.vector.memset(v_sb[:, :, :, 64:65], 1.0)
    qt_sb = res.tile([P, NF, SQ], BF16)
    ot_sb = res.tile([P, NF, SQ], BF16)      # concat.T (normalized)

    lnc = ctx.enter_context(tc.tile_pool(name="lnc", bufs=1))
    scale_sb = lnc.tile([P, 2, 512], F32)
    nc.sync.dma_start(scale_sb, ins["scale_b"].rearrange("p (a b) -> p a b", a=2))
    offset_sb = lnc.tile([P, 2, 512], F32)
    nc.sync.dma_start(offset_sb, ins["offset_b"].rearrange("p (a b) -> p a b", a=2))

    with (
        tc.tile_pool(name="ktp", bufs=2) as ktp,
        tc.tile_pool(name="expp", bufs=4) as expp,
        tc.tile_pool(name="nrm", bufs=2) as nrm,
        tc.tile_pool(name="oo", bufs=2) as oop,
        tc.tile_pool(name="misc_ps", bufs=2, space="PSUM") as misc_ps,
        tc.tile_pool(name="sc_ps", bufs=2, space="PSUM") as sc_ps,
        tc.tile_pool(name="pv_ps", bufs=2, space="PSUM") as pv_ps,
    ):
        kt_tiles = [None] * NF

        for _ in range(40):
            wps_ = misc_ps.tile([P, 512], F32, tag="ps", name="warm")
            nc.tensor.matmul(wps_, lhsT=warm_b[:, 0:128], rhs=warm_b,
                             start=True, stop=True)

        # ---- projection work generator: each yield emits one PSUM
        # accumulation group (8 matmuls) + its PSUM->SBUF cast.
        def proj_work():
            for c in range(NF):
                # K chunk c: KT[c*128:(c+1)*128, :] into a fresh SBUF tile
                kt = ktp.tile([P, SK], BF16, tag="kt")
                kt_tiles[c] = kt
                for n in range(SK // 512):
                    ps = misc_ps.tile([P, 512], F32, tag="ps", name="kq")
                    for c2 in range(KD2):
                        nc.tensor.matmul(
                            ps,
                            lhsT=wk[:, c2, :, c * P:(c + 1) * P],
                            rhs=xk[:, c2, :, n * 512:(n + 1) * 512],
                            start=(c2 == 0),
                            stop=(c2 == KD2 - 1),
                            perf_mode=DR,
                        )
                    nc.vector.tensor_copy(kt[:, n * 512:(n + 1) * 512], ps)
                    yield
                # Q chunk c
                for n in range(SQ // 512):
                    ps = misc_ps.tile([P, 512], F32, tag="ps", name="kq")
                    for c2 in range(KD2):
                        nc.tensor.matmul(
                            ps,
                            lhsT=wq[:, c2, :, c * P:(c + 1) * P],
                            rhs=xq[:, c2, :, n * 512:(n + 1) * 512],
                            start=(c2 == 0),
                            stop=(c2 == KD2 - 1),
                            perf_mode=DR,
                        )
                    nc.vector.tensor_copy(qt_sb[:, c, n * 512:(n + 1) * 512], ps)
                    yield
                # V: first two chunks emit the f-halves early enough; V for
                # head-pair c is the f-slice [c*128:(c+1)*128] -> emit V at
                # pair granularity (free dim 128 in two 16-sk passes would be
                # slow) -> use 256-wide slices every other c.
                if c % 2 == 0:
                    g = c // 2
                    for sk in range(NSK):
                        ps = misc_ps.tile([P, 256], F32, tag="ps", name="vp")
                        for c2 in range(KD2):
                            nc.tensor.matmul(
                                ps,
                                lhsT=xv[:, c2, :, sk * P:(sk + 1) * P],
                                rhs=wv[:, c2, :, g * 256:(g + 1) * 256],
                                start=(c2 == 0),
                                stop=(c2 == KD2 - 1),
                                perf_mode=DR,
                            )
                        nc.vector.tensor_copy(
                            v_sb[:, sk, 4 * g:4 * g + 4, 0:64],
                            ps.rearrange("p (h e) -> p h e", h=4),
                        )
                        yield

        stream = proj_work()
        drained = False

        def pull(n):
            nonlocal drained
            if drained:
                return
            for _ in range(n):
                try:
                    next(stream)
                except StopIteration:
                    drained = True
                    return

        # startup: K0 (4 groups), Q0 (2 groups), V chunks for pair 0 get
        # pulled inside the first attention block.
        pull(6)

        # Per-block pull quotas keep the generator exactly one head-pair
        # chunk ahead of the attention block consuming it: emitting a
        # K-chunk cast too early would recycle a kt pool slot whose reader
        # scores are not yet emitted (wrong order -> scheduler deadlock).
        # Block t=2c+sq; gen block g = K_g, Q_g (+ V pairs {g, g+1} if g
        # even, 16 more groups). Quota for block t covers gen block t//2+1
        # split over its two sq halves.
        QUOTA = [16, 6, 11, 11, 3, 3, 11, 11, 3, 3, 11, 11, 3, 3, 0, 0]

        # ---- attention blocks ----
        # The PV accumulators are drained to SBUF with one quick copy as
        # soon as the block's accumulation stops, so the PSUM slots recycle
        # ~0.7us later and the PE never bubbles at block boundaries (a
        # bubble >3.4us re-throttles the PE clock to 1.2 GHz).  The whole
        # normalize chain then runs off SBUF on DVE+GpSimd, never touching
        # the PE.

        def emit_scores(c, sq, sk):
            sc = sc_ps.tile([P, 2, 512], F32, tag="sc", name="sc")
            for hh in range(2):
                base = hh * 64
                nc.tensor.matmul(
                    sc[:, hh, :],
                    lhsT=kt_tiles[c][base:base + 64, sk * P:(sk + 1) * P],
                    rhs=qt_sb[base:base + 64, c, sq * 512:(sq + 1) * 512],
                    start=True,
                    stop=True,
                )
            return sc

        sc_carry = None
        for c in range(NF):
            for sq in range(2):
                quota = QUOTA[2 * c + sq]
                done = 0
                ot_ps = [None, None]
                # scores(0) of this block were prefetched by the previous
                # block at its iter 14 so the ACT exp stream never breaks
                # across block boundaries.
                sc_prev = sc_carry if sc_carry is not None \
                    else emit_scores(c, sq, 0)
                sc_carry = None
                dve_sks = (7, 12) if quota <= 3 else ()
                for sk in range(NSK):
                    ex = expp.tile([P, 2, 512], BF16, tag="ex", name="ex")
                    if sk in dve_sks:
                        nc.vector.tensor_scalar(
                            ex.bitcast(I16), sc_prev, SCHRA_A, SCHRA_B,
                            ALU.mult, ALU.add)
                    else:
                        nc.scalar.activation(ex, sc_prev, AF.Exp,
                                             scale=EXP_SCALE)
                    if sk + 1 < NSK:
                        sc_prev = emit_scores(c, sq, sk + 1)
                    if sk == NSK - 2 and (c, sq) != (NF - 1, 1):
                        nc_, nsq = (c, 1) if sq == 0 else (c + 1, 0)
                        sc_carry = emit_scores(nc_, nsq, 0)
                    # interleave projection work, front-loaded at the block
                    # start so the PE has independent work queued while the
                    # previous block's normalize chain (DVE) produces the
                    # reciprocals that gate bc and the pv slot recycling.
                    want = max(min(2, quota), (sk + 1) * quota // NSK)
                    if want > done:
                        pull(want - done)
                        done = want
                    if sk == 0:
                        for hh in range(2):
                            ot_ps[hh] = pv_ps.tile(
                                [65, 512], F32, tag="pv", name="otp")
                    for hh in range(2):
                        nc.tensor.matmul(
                            ot_ps[hh],
                            lhsT=v_sb[:, sk, 2 * c + hh, :],
                            rhs=ex[:, hh, :],
                            start=(sk == 0),
                            stop=(sk == NSK - 1),
                        )

                for hh in range(2):
                    pv = ot_ps[hh]
                    # quick PSUM->SBUF drain releases the accumulator bank
                    pvs = nrm.tile([65, 512], F32, tag="pvs")
                    nc.vector.tensor_copy(pvs, pv)
                    # Denominator to partition 0 (shifted copies are the
                    # one partition-moving DVE op that works); the custom
                    # reciprocal uop silently ignores nonzero bases.  The
                    # 1/WS unscale folds into the post-reciprocal scale.
                    den = nrm.tile([1, 512], F32, tag="den")
                    nc.vector.tensor_copy(den, pvs[64:65, :])
                    rden = nrm.tile([1, 512], F32, tag="rden")
                    nc.vector.reciprocal_approx_fast(rden, den)
                    rcb = nrm.tile([1, 512], F32, tag="rcb")
                    nc.vector.tensor_scalar(
                        rcb, rden, 1.0 / WS, None, ALU.mult)
                    bcs = nrm.tile([64, 512], F32, tag="bcs")
                    nc.gpsimd.partition_broadcast(bcs, rcb, channels=64)
                    if hh == 0:
                        nc.vector.tensor_tensor(
                            ot_sb[0:64, c, sq * 512:(sq + 1) * 512],
                            pvs[0:64, :], bcs, ALU.mult)
                    else:
                        oo = oop.tile([64, 512], BF16, tag="oo")
                        nc.vector.tensor_tensor(
                            oo, pvs[0:64, :], bcs, ALU.mult)
                        nc.vector.tensor_copy(
                            ot_sb[64:128, c, sq * 512:(sq + 1) * 512],
                            oo)
        pull(1000)  # drain any remaining projection work (paranoia)

    # ---------------- output projection + residual + layernorm -------------
    with (
        tc.tile_pool(name="qres", bufs=3) as qrp,
        tc.tile_pool(name="lnw", bufs=3) as lnw,
        tc.tile_pool(name="stat", bufs=8) as stp,
        tc.tile_pool(name="fps", bufs=4, space="PSUM") as fps,
    ):
        qres_r = ins["qres16"]
        for sq in range(SQ // P):  # 8 query chunks of 128
            qr = qrp.tile([P, 2, 512], BF16, tag="qr")
            nc.sync.dma_start(
                qr,
                qres_r[sq * P:(sq + 1) * P, :].rearrange(
                    "p (a b) -> p a b", a=2),
            )
            fp = fps.tile([P, 2, 512], F32, tag="fp")
            for dh in range(2):
                for f in range(NF):
                    nc.tensor.matmul(
                        fp[:, dh, :],
                        lhsT=ot_sb[:, f, sq * P:(sq + 1) * P],
                        rhs=wp[:, f, dh * 512:(dh + 1) * 512],
                        start=(f == 0),
                        stop=False,
                    )
                # residual rides the accumulation as an identity matmul
                nc.tensor.matmul(
                    fp[:, dh, :],
                    lhsT=ident,
                    rhs=qr[:, dh, :],
                    start=False,
                    stop=True,
                )
            stats = stp.tile([P, 2, 6], F32, tag="st")
            for gsub in range(2):
                nc.vector.bn_stats(stats[:, gsub, :], fp[:, gsub, :])
            mv = stp.tile([P, 2], F32, tag="mv")
            nc.vector.bn_aggr(mv, stats)
            stdt = stp.tile([P, 1], F32, tag="sd")
            nc.scalar.activation(stdt, mv[:, 1:2], AF.Sqrt,
                                 scale=float(D) / float(D - 1))
            nc.vector.tensor_scalar_add(stdt, stdt, EPS)
            rstd = stp.tile([P, 1], F32, tag="rs")
            nc.vector.reciprocal(rstd, stdt)
            # (x - mean)*gamma fused on DVE straight off PSUM, *rstd
            # per-partition, +beta on the otherwise idle GpSimd.
            xg = lnw.tile([P, 2, 512], F32, tag="xg")
            nc.vector.scalar_tensor_tensor(
                xg, fp, mv[:, 0:1], scale_sb, ALU.subtract, ALU.mult)
            xn = lnw.tile([P, 2, 512], F32, tag="xn")
            nc.vector.tensor_scalar(xn, xg, rstd, None, ALU.mult)
            nc.gpsimd.tensor_add(xn, xn, offset_sb)
            xnv = xn.rearrange("p a b -> p (a b)").rearrange(
                "p (a b) -> p a b", a=4)
            dst = out_ap[sq * P:(sq + 1) * P, :].rearrange(
                "p (a b) -> p a b", a=4)
            for a in range(4):
                nc.sync.dma_start(dst[:, a, :], xnv[:, a, :])


def build_program():
    nc = bacc.Bacc("TRN2", debug=False, target_bir_lowering=False)
    shapes = {
        "qT": ([D, SQ], FP8), "kT": ([D, SK], FP8), "vT": ([D, SK], FP8),
        "qres16": ([SQ, D], BF16), "ident": ([P, P], BF16),
        "wqT": ([D, F], FP8), "wkT": ([D, F], FP8), "wvT": ([D, F], FP8),
        "wpT": ([F, D], BF16),
        "scale_b": ([P, D], F32), "offset_b": ([P, D], F32),
    }
    ins = {k: nc.dram_tensor(k, shp, dt, kind="ExternalInput").ap()
           for k, (shp, dt) in shapes.items()}
    out = nc.dram_tensor("out", [SQ, D], F32, kind="ExternalOutput").ap()
    with tile.TileContext(nc) as tc:
        _mha_kernel(tc, out, ins)
    nc.compile()
    return nc


_PROGRAM = None


def _get_program():
    global _PROGRAM
    if _PROGRAM is None:
        _PROGRAM = build_program()
    return _PROGRAM


def make_in_maps(q, k, v, Wq, Wk, Wv, Wp, scale, offset):
    import ml_dtypes
    f = np.float32
    bf = ml_dtypes.bfloat16
    f8 = ml_dtypes.float8_e4m3
    q = np.asarray(q, f)
    k8 = np.asarray(k, f).astype(f8)
    v8 = np.asarray(v, f).astype(f8)
    q8 = q.astype(f8)
    q16 = q.astype(bf)
    wqT = np.ascontiguousarray(
        (np.asarray(Wq, f) * WS).transpose(2, 0, 1).reshape(D, F).astype(f8))
    wkT = np.ascontiguousarray(
        (np.asarray(Wk, f) * WS).transpose(2, 0, 1).reshape(D, F).astype(f8))
    wvT = np.ascontiguousarray(
        (np.asarray(Wv, f) * WS).transpose(2, 0, 1).reshape(D, F).astype(f8))
    wpT = np.ascontiguousarray(np.asarray(Wp, f).T.astype(bf))
    scale_b = np.ascontiguousarray(
        np.broadcast_to(np.asarray(scale, f), (P, D)))
    offset_b = np.ascontiguousarray(
        np.broadcast_to(np.asarray(offset, f), (P, D)))
    ident = np.ascontiguousarray(np.eye(P, dtype=np.float32).astype(bf))
    in_maps = []
    for c in range(N_CORES):
        b, half = divmod(c, 2)
        sl = slice(half * SQ, (half + 1) * SQ)
        in_maps.append({
            "qT": np.ascontiguousarray(q8[b, sl].T),
            "qres16": np.ascontiguousarray(q16[b, sl]),
            "ident": ident,
            "kT": np.ascontiguousarray(k8[b].T),
            "vT": np.ascontiguousarray(v8[b].T),
            "wqT": wqT, "wkT": wkT, "wvT": wvT, "wpT": wpT,
            "scale_b": scale_b, "offset_b": offset_b,
        })
    return in_maps


def kernel(q, k, v, Wq, Wk, Wv, Wp, scale, offset):
    global LAST_RESULT
    in_maps = make_in_maps(q, k, v, Wq, Wk, Wv, Wp, scale, offset)
    nc = _get_program()
    res = run_bass_kernel_spmd(nc, in_maps, list(range(N_CORES)))
    LAST_RESULT = res
    out = np.empty((B, S, D), np.float32)
    for c in range(N_CORES):
        b, half = divmod(c, 2)
        out[b, half * SQ:(half + 1) * SQ] = res.results[c]["out"]
    return out


# revision 30
# speedup vs baseline: 1.0036x; 1.0036x over previous
"""Multi-head attention + out-proj + residual + LayerNorm on 8 trn2 cores.

Sharding: (batch, seq-half) -> 8 shards, collective-free. Each core gets
transposed activations plus shared weights and computes its full
[1024, 1024] output block.  Measured ~421us (baseline 741us).

Single fused pipeline, everything SBUF-resident (no DRAM staging):
  - qT/kT/vT and Wq/Wk/Wv stored fp8e4, weights pre-scaled x16 on the
    host (fp8e4m3 subnormal avoidance); projections run as fp8 DoubleRow
    matmuls (pair-interleaved [p, c2, j, :] layout, contraction 256/pass).
  - projection work (K_c, Q_c, V chunks) is emitted through a generator
    and interleaved into the attention inner loop, paced one head-pair
    chunk ahead of consumption (emitting a chunk too early recycles a kt
    pool slot whose reader scores are not yet emitted -> scheduler
    deadlock; too late starves the PE).
  - per head-pair chunk c: scoresT[k,q] = KT_c.T @ QT_c (the two heads
    ride disjoint PE row groups via tile_position), exp on ACT with the
    1/(sqrt(D)*256) fold, OT[dv+1,q] += [V|1].T @ exp (row 64 = softmax
    denominator).  Scores for the next block's chunk 0 are prefetched at
    iter 14 so the ACT stream does not break at block boundaries.
  - normalize, entirely off the PE: one quick PSUM->SBUF drain frees the
    accumulator bank (~0.7us, keeps the PE from bubbling >3.4us at block
    boundaries, which would re-throttle the PE clock to 1.2 GHz);
    denominator hops to partition 0 via a partition-shifted DVE copy
    (the one shifted op that works; reciprocal_approx_fast silently
    ignores nonzero base partitions), reciprocal_approx_fast, 1/16
    unscale, gpsimd partition_broadcast, TT multiply.  hh=1 lands on
    partitions 64:127 via a second shifted copy.
  - final: out = LN(ot_sb.T @ Wp + qres)*scale + offset; the residual is
    accumulated into the out-proj PSUM via an identity matmul on a bf16
    copy of q, and (x-mean)*gamma is fused in one scalar_tensor_tensor.

Hardware landmines found on the way (verified by experiment):
  - a 1-contraction (32,64)@(64,0) broadcast matmul emitted inside an
    open PSUM accumulation window corrupts other in-flight matmul groups;
  - reciprocal_approx_fast returns garbage for APs based at partition!=0;
  - DVE tensor_tensor with mismatched operand base partitions does not
    compile; partition-shifted tensor_copy works and is bit-exact.
"""

import os
from contextlib import ExitStack

import numpy as np

import concourse.bass as bass
import concourse.tile as tile
from concourse import bacc, mybir
from concourse._compat import with_exitstack
from concourse.bass_utils import run_bass_kernel_spmd

B, S, D = 4, 2048, 1024
H, DK, DV = 16, 64, 64
F = H * DV            # 1024 flattened head dim (== H*DK)
N_CORES = 8
SQ = S // 2           # 1024 queries per core
SK = S                # 2048 keys per core
P = 128
KD = D // P           # 8 contraction chunks over d_model
NF = F // P           # 8 head-pair chunks
NSK = SK // P         # 16 key chunks
TEMP = float(np.sqrt(D))
WS = 16.0             # host-side weight scale (fp8 subnormal avoidance)
EXP_SCALE = 1.0 / (TEMP * WS * WS)
EPS = 1e-9

F32 = mybir.dt.float32
BF16 = mybir.dt.bfloat16
FP8 = mybir.dt.float8e4
I16 = mybir.dt.int16
DR = mybir.MatmulPerfMode.DoubleRow
# Schraudolph fast-exp constants (bf16 bit trick): round(x*128/ln2 + bias)
# interpreted as bf16 bits ~= exp(x) to ~2-4% -- plenty within tolerance,
# used on a few chunks per block to offload the ACT engine.
SCHRA_A = EXP_SCALE * 128.0 / float(np.log(2.0))
SCHRA_B = 127.0 * 128.0 - 4.5
DVE_EXP_SKS = ()

LAST_RESULT = None


@with_exitstack
def _mha_kernel(ctx: ExitStack, tc: tile.TileContext, out_ap, ins):
    nc = tc.nc
    AF = mybir.ActivationFunctionType
    ALU = mybir.AluOpType

    g_const = ctx.enter_context(tc.tile_pool(name="gconst", bufs=1))
    ones_sb = g_const.tile([P, 64], BF16)
    nc.vector.memset(ones_sb, 1.0)
    # warm-up fodder: keeps the PE HAM busy through the initial DMA wait
    # (else the first ~20us of real matmuls run at 1.2 GHz) and triggers
    # the ACT exp table load (~2.7us) before the first real exp.
    warm_b = g_const.tile([P, 512], BF16)
    nc.vector.memset(warm_b, 0.125)
    warm_f = g_const.tile([P, 64], F32)
    nc.vector.memset(warm_f, 0.0)
    warm_e = g_const.tile([P, 64], BF16)
    nc.scalar.activation(warm_e, warm_f, mybir.ActivationFunctionType.Exp,
                         scale=1.0)

    res = ctx.enter_context(tc.tile_pool(name="resident", bufs=1))
    # inputs (fp8) in DoubleRow pair-interleaved layout: D-row
    # c2*256 + 2p + j lives at [p, c2, j, :]; loaded in pieces so early
    # matmuls unblock.
    KD2 = KD // 2
    xq = res.tile([P, KD2, 2, SQ], FP8)
    xk = res.tile([P, KD2, 2, SK], FP8)
    xv = res.tile([P, KD2, 2, SK], FP8)
    wq = res.tile([P, KD2, 2, F], FP8)
    wk = res.tile([P, KD2, 2, F], FP8)
    wv = res.tile([P, KD2, 2, F], FP8)
    wp = res.tile([P, NF, D], BF16)
    xq_r = ins["qT"].rearrange("(c p j) s -> p c j s", c=KD2, j=2)
    xk_r = ins["kT"].rearrange("(c p j) s -> p c j s", c=KD2, j=2)
    xv_r = ins["vT"].rearrange("(c p j) s -> p c j s", c=KD2, j=2)
    wq_r = ins["wqT"].rearrange("(c p j) f -> p c j f", c=KD2, j=2)
    wk_r = ins["wkT"].rearrange("(c p j) f -> p c j f", c=KD2, j=2)
    wv_r = ins["wvT"].rearrange("(c p j) f -> p c j f", c=KD2, j=2)
    # K-proj inputs first (dma_starts spread round-robin over the DMA
    # queues land in parallel), then Q, V, and the late-needed wp.
    for c2 in range(KD2):
        for j in range(2):
            nc.sync.dma_start(wk[:, c2, j, :], wk_r[:, c2, j, :])
            nc.sync.dma_start(xk[:, c2, j, 0:SK // 2],
                              xk_r[:, c2, j, 0:SK // 2])
            nc.sync.dma_start(xk[:, c2, j, SK // 2:],
                              xk_r[:, c2, j, SK // 2:])
    for c2 in range(KD2):
        for j in range(2):
            nc.sync.dma_start(wq[:, c2, j, :], wq_r[:, c2, j, :])
            nc.sync.dma_start(xq[:, c2, j, :], xq_r[:, c2, j, :])
    for c2 in range(KD2):
        for j in range(2):
            nc.sync.dma_start(wv[:, c2, j, :], wv_r[:, c2, j, :])
            nc.sync.dma_start(xv[:, c2, j, :], xv_r[:, c2, j, :])
    wp_r = ins["wpT"].rearrange("(c p) f -> p c f", p=P)
    for f_ in range(NF):
        nc.sync.dma_start(wp[:, f_, :], wp_r[:, f_, :])
    ident = res.tile([P, P], BF16)
    nc.sync.dma_start(ident, ins["ident"])

    # persistent activations
    v_sb = res.tile([P, NSK, H, 65], BF16)   # [V | ones] per head
    nc.vector.memset(v_sb[:, :, :, 64:65], 1.0)
    qt_sb = res.tile([P, NF, SQ], BF16)
    ot_sb = res.tile([P, NF, SQ], BF16)      # concat.T (normalized)

    lnc = ctx.enter_context(tc.tile_pool(name="lnc", bufs=1))
    scale_sb = lnc.tile([P, 2, 512], F32)
    nc.sync.dma_start(scale_sb, ins["scale_b"].rearrange("p (a b) -> p a b", a=2))
    offset_sb = lnc.tile([P, 2, 512], F32)
    nc.sync.dma_start(offset_sb, ins["offset_b"].rearrange("p (a b) -> p a b", a=2))

    with (
        tc.tile_pool(name="ktp", bufs=2) as ktp,
        tc.tile_pool(name="expp", bufs=4) as expp,
        tc.tile_pool(name="nrm", bufs=2) as nrm,
        tc.tile_pool(name="oo", bufs=2) as oop,
        tc.tile_pool(name="misc_ps", bufs=2, space="PSUM") as misc_ps,
        tc.tile_pool(name="sc_ps", bufs=2, space="PSUM") as sc_ps,
        tc.tile_pool(name="pv_ps", bufs=2, space="PSUM") as pv_ps,
    ):
        kt_tiles = [None] * NF

        # ---- projection work generator: each yield emits one PSUM
        # accumulation group (8 matmuls) + its PSUM->SBUF cast.
        def proj_work():
            for c in range(NF):
                # K chunk c: KT[c*128:(c+1)*128, :] into a fresh SBUF tile
                kt = ktp.tile([P, SK], BF16, tag="kt")
                kt_tiles[c] = kt
                for n in range(SK // 512):
                    ps = misc_ps.tile([P, 512], F32, tag="ps", name="kq")
                    for c2 in range(KD2):
                        nc.tensor.matmul(
                            ps,
                            lhsT=wk[:, c2, :, c * P:(c + 1) * P],
                            rhs=xk[:, c2, :, n * 512:(n + 1) * 512],
                            start=(c2 == 0),
                            stop=(c2 == KD2 - 1),
                            perf_mode=DR,
                        )
                    nc.vector.tensor_copy(kt[:, n * 512:(n + 1) * 512], ps)
                    yield
                # Q chunk c
                for n in range(SQ // 512):
                    ps = misc_ps.tile([P, 512], F32, tag="ps", name="kq")
                    for c2 in range(KD2):
                        nc.tensor.matmul(
                            ps,
                            lhsT=wq[:, c2, :, c * P:(c + 1) * P],
                            rhs=xq[:, c2, :, n * 512:(n + 1) * 512],
                            start=(c2 == 0),
                            stop=(c2 == KD2 - 1),
                            perf_mode=DR,
                        )
                    nc.vector.tensor_copy(qt_sb[:, c, n * 512:(n + 1) * 512], ps)
                    yield
                # V: first two chunks emit the f-halves early enough; V for
                # head-pair c is the f-slice [c*128:(c+1)*128] -> emit V at
                # pair granularity (free dim 128 in two 16-sk passes would be
                # slow) -> use 256-wide slices every other c.
                if c % 2 == 0:
                    g = c // 2
                    for sk in range(NSK):
                        ps = misc_ps.tile([P, 256], F32, tag="ps", name="vp")
                        for c2 in range(KD2):
                            nc.tensor.matmul(
                                ps,
                                lhsT=xv[:, c2, :, sk * P:(sk + 1) * P],
                                rhs=wv[:, c2, :, g * 256:(g + 1) * 256],
                                start=(c2 == 0),
                                stop=(c2 == KD2 - 1),
                                perf_mode=DR,
                            )
                        nc.vector.tensor_copy(
                            v_sb[:, sk, 4 * g:4 * g + 4, 0:64],
                            ps.rearrange("p (h e) -> p h e", h=4),
                        )
                        yield

        stream = proj_work()
        drained = False

        def pull(n):
            nonlocal drained
            if drained:
                return
            for _ in range(n):
                try:
                    next(stream)
                except StopIteration:
                    drained = True
                    return

        # startup: K0 (4 groups), Q0 (2 groups), V chunks for pair 0 get
        # pulled inside the first attention block.
        pull(6)

        # Per-block pull quotas keep the generator exactly one head-pair
        # chunk ahead of the attention block consuming it: emitting a
        # K-chunk cast too early would recycle a kt pool slot whose reader
        # scores are not yet emitted (wrong order -> scheduler deadlock).
        # Block t=2c+sq; gen block g = K_g, Q_g (+ V pairs {g, g+1} if g
        # even, 16 more groups). Quota for block t covers gen block t//2+1
        # split over its two sq halves.
        QUOTA = [16, 6, 11, 11, 3, 3, 11, 11, 3, 3, 11, 11, 3, 3, 0, 0]

        # ---- attention blocks ----
        # The PV accumulators are drained to SBUF with one quick copy as
        # soon as the block's accumulation stops, so the PSUM slots recycle
        # ~0.7us later and the PE never bubbles at block boundaries (a
        # bubble >3.4us re-throttles the PE clock to 1.2 GHz).  The whole
        # normalize chain then runs off SBUF on DVE+GpSimd, never touching
        # the PE.

        def emit_scores(c, sq, sk):
            sc = sc_ps.tile([P, 2, 512], F32, tag="sc", name="sc")
            for hh in range(2):
                base = hh * 64
                nc.tensor.matmul(
                    sc[:, hh, :],
                    lhsT=kt_tiles[c][base:base + 64, sk * P:(sk + 1) * P],
                    rhs=qt_sb[base:base + 64, c, sq * 512:(sq + 1) * 512],
                    start=True,
                    stop=True,
                )
            return sc

        sc_carry = None
        for c in range(NF):
            for sq in range(2):
                quota = QUOTA[2 * c + sq]
                done = 0
                ot_ps = [None, None]
                # scores(0) of this block were prefetched by the previous
                # block at its iter 14 so the ACT exp stream never breaks
                # across block boundaries.
                sc_prev = sc_carry if sc_carry is not None \
                    else emit_scores(c, sq, 0)
                sc_carry = None
                dve_sks = (7, 12) if quota <= 3 else ()
                for sk in range(NSK):
                    ex = expp.tile([P, 2, 512], BF16, tag="ex", name="ex")
                    if sk in dve_sks:
                        nc.vector.tensor_scalar(
                            ex.bitcast(I16), sc_prev, SCHRA_A, SCHRA_B,
                            ALU.mult, ALU.add)
                    else:
                        nc.scalar.activation(ex, sc_prev, AF.Exp,
                                             scale=EXP_SCALE)
                    if sk + 1 < NSK:
                        sc_prev = emit_scores(c, sq, sk + 1)
                    if sk == NSK - 2 and (c, sq) != (NF - 1, 1):
                        nc_, nsq = (c, 1) if sq == 0 else (c + 1, 0)
                        sc_carry = emit_scores(nc_, nsq, 0)
                    # interleave projection work, front-loaded at the block
                    # start so the PE has independent work queued while the
                    # previous block's normalize chain (DVE) produces the
                    # reciprocals that gate bc and the pv slot recycling.
                    want = max(min(2, quota), (sk + 1) * quota // NSK)
                    if want > done:
                        pull(want - done)
                        done = want
                    if sk == 0:
                        for hh in range(2):
                            ot_ps[hh] = pv_ps.tile(
                                [65, 512], F32, tag="pv", name="otp")
                    for hh in range(2):
                        nc.tensor.matmul(
                            ot_ps[hh],
                            lhsT=v_sb[:, sk, 2 * c + hh, :],
                            rhs=ex[:, hh, :],
                            start=(sk == 0),
                            stop=(sk == NSK - 1),
                        )

                for hh in range(2):
                    pv = ot_ps[hh]
                    # quick PSUM->SBUF drain releases the accumulator bank
                    pvs = nrm.tile([65, 512], F32, tag="pvs")
                    nc.vector.tensor_copy(pvs, pv)
                    # Denominator to partition 0 (shifted copies are the
                    # one partition-moving DVE op that works); the custom
                    # reciprocal uop silently ignores nonzero bases.  The
                    # 1/WS unscale folds into the post-reciprocal scale.
                    den = nrm.tile([1, 512], F32, tag="den")
                    nc.vector.tensor_copy(den, pvs[64:65, :])
                    rden = nrm.tile([1, 512], F32, tag="rden")
                    nc.vector.reciprocal_approx_fast(rden, den)
                    rcb = nrm.tile([1, 512], F32, tag="rcb")
                    nc.vector.tensor_scalar(
                        rcb, rden, 1.0 / WS, None, ALU.mult)
                    bcs = nrm.tile([64, 512], F32, tag="bcs")
                    nc.gpsimd.partition_broadcast(bcs, rcb, channels=64)
                    if hh == 0:
                        nc.vector.tensor_tensor(
                            ot_sb[0:64, c, sq * 512:(sq + 1) * 512],
                            pvs[0:64, :], bcs, ALU.mult)
                    else:
                        oo = oop.tile([64, 512], BF16, tag="oo")
                        nc.vector.tensor_tensor(
                            oo, pvs[0:64, :], bcs, ALU.mult)
                        nc.vector.tensor_copy(
                            ot_sb[64:128, c, sq * 512:(sq + 1) * 512],
                            oo)
        pull(1000)  # drain any remaining projection work (paranoia)

    # ---------------- output projection + residual + layernorm -------------
    with (
        tc.tile_pool(name="qres", bufs=3) as qrp,
        tc.tile_pool(name="lnw", bufs=3) as lnw,
        tc.tile_pool(name="stat", bufs=8) as stp,
        tc.tile_pool(name="fps", bufs=4, space="PSUM") as fps,
    ):
        qres_r = ins["qres16"]
        for sq in range(SQ // P):  # 8 query chunks of 128
            qr = qrp.tile([P, 2, 512], BF16, tag="qr")
            nc.sync.dma_start(
                qr,
                qres_r[sq * P:(sq + 1) * P, :].rearrange(
                    "p (a b) -> p a b", a=2),
            )
            fp = fps.tile([P, 2, 512], F32, tag="fp")
            for dh in range(2):
                for f in range(NF):
                    nc.tensor.matmul(
                        fp[:, dh, :],
                        lhsT=ot_sb[:, f, sq * P:(sq + 1) * P],
                        rhs=wp[:, f, dh * 512:(dh + 1) * 512],
                        start=(f == 0),
                        stop=False,
                    )
                # residual rides the accumulation as an identity matmul
                nc.tensor.matmul(
                    fp[:, dh, :],
                    lhsT=ident,
                    rhs=qr[:, dh, :],
                    start=False,
                    stop=True,
                )
            stats = stp.tile([P, 2, 6], F32, tag="st")
            for gsub in range(2):
                nc.vector.bn_stats(stats[:, gsub, :], fp[:, gsub, :])
            mv = stp.tile([P, 2], F32, tag="mv")
            nc.vector.bn_aggr(mv, stats)
            stdt = stp.tile([P, 1], F32, tag="sd")
            nc.scalar.activation(stdt, mv[:, 1:2], AF.Sqrt,
                                 scale=float(D) / float(D - 1))
            nc.vector.tensor_scalar_add(stdt, stdt, EPS)
            rstd = stp.tile([P, 1], F32, tag="rs")
            nc.vector.reciprocal(rstd, stdt)
            # (x - mean)*gamma fused on DVE straight off PSUM, *rstd
            # per-partition, +beta on the otherwise idle GpSimd.
            xg = lnw.tile([P, 2, 512], F32, tag="xg")
            nc.vector.scalar_tensor_tensor(
                xg, fp, mv[:, 0:1], scale_sb, ALU.subtract, ALU.mult)
            xn = lnw.tile([P, 2, 512], F32, tag="xn")
            nc.vector.tensor_scalar(xn, xg, rstd, None, ALU.mult)
            nc.gpsimd.tensor_add(xn, xn, offset_sb)
            xnv = xn.rearrange("p a b -> p (a b)").rearrange(
                "p (a b) -> p a b", a=4)
            dst = out_ap[sq * P:(sq + 1) * P, :].rearrange(
                "p (a b) -> p a b", a=4)
            for a in range(4):
                nc.sync.dma_start(dst[:, a, :], xnv[:, a, :])


def build_program():
    nc = bacc.Bacc("TRN2", debug=False, target_bir_lowering=False)
    shapes = {
        "qT": ([D, SQ], FP8), "kT": ([D, SK], FP8), "vT": ([D, SK], FP8),
        "qres16": ([SQ, D], BF16), "ident": ([P, P], BF16),
        "wqT": ([D, F], FP8), "wkT": ([D, F], FP8), "wvT": ([D, F], FP8),
        "wpT": ([F, D], BF16),
        "scale_b": ([P, D], F32), "offset_b": ([P, D], F32),
    }
    ins = {k: nc.dram_tensor(k, shp, dt, kind="ExternalInput").ap()
           for k, (shp, dt) in shapes.items()}
    out = nc.dram_tensor("out", [SQ, D], F32, kind="ExternalOutput").ap()
    with tile.TileContext(nc) as tc:
        _mha_kernel(tc, out, ins)
    nc.compile()
    return nc


_PROGRAM = None


def _get_program():
    global _PROGRAM
    if _PROGRAM is None:
        _PROGRAM = build_program()
    return _PROGRAM


def make_in_maps(q, k, v, Wq, Wk, Wv, Wp, scale, offset):
    import ml_dtypes
    f = np.float32
    bf = ml_dtypes.bfloat16
    f8 = ml_dtypes.float8_e4m3
    q = np.asarray(q, f)
    k8 = np.asarray(k, f).astype(f8)
    v8 = np.asarray(v, f).astype(f8)
    q8 = q.astype(f8)
    q16 = q.astype(bf)
    wqT = np.ascontiguousarray(
        (np.asarray(Wq, f) * WS).transpose(2, 0, 1).reshape(D, F).astype(f8))
    wkT = np.ascontiguousarray(
        (np.asarray(Wk, f) * WS).transpose(2, 0, 1).reshape(D, F).astype(f8))
    wvT = np.ascontiguousarray(
        (np.asarray(Wv, f) * WS).transpose(2, 0, 1).reshape(D, F).astype(f8))
    wpT = np.ascontiguousarray(np.asarray(Wp, f).T.astype(bf))
    scale_b = np.ascontiguousarray(
        np.broadcast_to(np.asarray(scale, f), (P, D)))
    offset_b = np.ascontiguousarray(
        np.broadcast_to(np.asarray(offset, f), (P, D)))
    ident = np.ascontiguousarray(np.eye(P, dtype=np.float32).astype(bf))
    in_maps = []
    for c in range(N_CORES):
        b, half = divmod(c, 2)
        sl = slice(half * SQ, (half + 1) * SQ)
        in_maps.append({
            "qT": np.ascontiguousarray(q8[b, sl].T),
            "qres16": np.ascontiguousarray(q16[b, sl]),
            "ident": ident,
            "kT": np.ascontiguousarray(k8[b].T),
            "vT": np.ascontiguousarray(v8[b].T),
            "wqT": wqT, "wkT": wkT, "wvT": wvT, "wpT": wpT,
            "scale_b": scale_b, "offset_b": offset_b,
        })
    return in_maps


def kernel(q, k, v, Wq, Wk, Wv, Wp, scale, offset):
    global LAST_RESULT
    in_maps = make_in_maps(q, k, v, Wq, Wk, Wv, Wp, scale, offset)
    nc = _get_program()
    res = run_bass_kernel_spmd(nc, in_maps, list(range(N_CORES)))
    LAST_RESULT = res
    out = np.empty((B, S, D), np.float32)
    for c in range(N_CORES):
        b, half = divmod(c, 2)
        out[b, half * SQ:(half + 1) * SQ] = res.results[c]["out"]
    return out
